# revision 6
# baseline (speedup 1.0000x reference)
"""Bilinear interpolation kernel for Trainium2 (8 NeuronCores, SPMD).

Strategy — "window-shared fat gathers":
  The per-point random gather is the bottleneck: SWDGE indirect DMA costs
  ~1-1.5us PER INSTRUCTION (128 descriptors max, one per partition), so the
  old one-16B-descriptor-per-point scheme is pinned at N/128 instructions.
  Instead:

  * Device builds a bf16 QUAD table: quad[f] = the 4 bilinear corners of
    flat position f = y*4096+x, 8 bytes each.  256B "windows" of the table
    cover 32 consecutive quads.
  * Host sorts each core's points by window id and packs up to PAD=8 points
    per occupied window into one "slot".  One indirect-DMA descriptor
    fetches one 256B window per partition -> 128 windows (up to 1024
    points) per instruction: ~4.1k gather instructions/core instead of 16k.
  * Each point selects its quad (position f mod 32 inside the window) on
    the Vector engine with a 5-level binary select tree (tensor_copy +
    copy_predicated with host-provided u8 mask planes), then the usual
    fp32 weight/blend/mask math.
  * Host un-permutes the outputs.

  Weights, validity and the blend are computed on-device in fp32 exactly as
  the reference; only the 4 corner values pass through bf16 (rel err ~4e-3,
  well inside the 2e-2 gate).
"""
import sys

sys.path.insert(0, "/opt/trn_rl_repo")

from contextlib import ExitStack

import numpy as np

import concourse.bass as bass
import concourse.mybir as mybir
from concourse.bass_utils import run_bass_kernel_spmd

H = W = 4096
N = 16777216
NCORES = 8
NPC = N // NCORES            # points per core (2_097_152)
PP = 128                     # partitions

QW = 32                      # quads per 256B window (bf16 quad = 8B)
PAD = 8                      # point positions per window slot
SPC = 32                     # slots per partition per chunk
PPC = SPC * PAD              # point columns per partition per chunk (256)
SLOTC = PP * SPC             # slots per chunk (4096)
NCHUNK = 129
NSLOT = NCHUNK * SLOTC       # 528_384 slots (mean occupancy ~525.8k)
NFLAT = (H - 1) * W          # quad table entries (16_773_120)
NWIN = NFLAT // QW           # windows (524_160)
MAGIC = 8388608.0            # 2^23 fp32 round-to-int magic

f32 = mybir.dt.float32
bf16 = mybir.dt.bfloat16
i32 = mybir.dt.int32
u8 = mybir.dt.uint8


def build_nc(reps=1):
    nc = bass.Bass()
    x_d = nc.declare_dram_parameter("x", [H, W], f32, isOutput=False)
    widx_d = nc.declare_dram_parameter("widx", [NCHUNK, PP, SPC], i32, isOutput=False)
    xq_d = nc.declare_dram_parameter("xq", [NCHUNK, PP, PPC], f32, isOutput=False)
    yq_d = nc.declare_dram_parameter("yq", [NCHUNK, PP, PPC], f32, isOutput=False)
    m_d = nc.declare_dram_parameter("m", [NCHUNK, 5, PP, PPC], u8, isOutput=False)
    val_d = nc.declare_dram_parameter("values", [NCHUNK, PP, PPC], f32, isOutput=True)
    vld_d = nc.declare_dram_parameter("valid", [NCHUNK, PP, PPC], u8, isOutput=True)
    quads_d = nc.dram_tensor("quads", [NWIN, QW * 4], bf16)

    es = ExitStack()
    with (
        nc.semaphore("p_in") as p_in,      # build: img tiles loaded
        nc.semaphore("p_cp") as p_cp,      # build: interleave done (1/iter)
        nc.semaphore("p_out") as p_out,    # build: quad tile stored
        nc.semaphore("cin") as cin,        # chunk inputs loaded (64/chunk)
        nc.semaphore("gdone") as gdone,    # gathers done (16/chunk)
        nc.semaphore("vdone") as vdone,    # tree L1 done -> G buf free (1/chunk)
        nc.semaphore("bdone") as bdone,    # blend done (1/chunk)
        nc.semaphore("odone") as odone,    # outputs stored (32/chunk)
        nc.Block() as block,
        es,
    ):
        def sb(name, shape, dt):
            return es.enter_context(nc.sbuf_tensor(name, shape, dt))

        HB = W // 2                     # build column half (2048)
        # build tiles
        ia = [sb(f"ia{b}", [PP, HB + 1], f32) for b in range(2)]
        ib = [sb(f"ib{b}", [PP, HB + 1], f32) for b in range(2)]
        qt = [sb(f"qt{b}", [PP, 4 * HB], bf16) for b in range(2)]
        # main loop tiles
        twidx = [sb(f"twidx{b}", [PP, SPC], i32) for b in range(2)]
        txq = [sb(f"txq{b}", [PP, PPC], f32) for b in range(2)]
        tyq = [sb(f"tyq{b}", [PP, PPC], f32) for b in range(2)]
        tm = [sb(f"tm{b}", [PP, 5 * PPC], u8) for b in range(2)]
        gt_ = [sb(f"g{b}", [PP, SPC * QW * 4], bf16) for b in range(2)]
        T1 = sb("T1", [PP, SPC * PAD * QW * 2], bf16)
        T2 = sb("T2", [PP, SPC * PAD * QW], bf16)
        T3 = sb("T3", [PP, SPC * PAD * QW // 2], bf16)
        T4 = sb("T4", [PP, SPC * PAD * QW // 4], bf16)
        T5 = sb("T5", [PP, SPC * PAD * 4], bf16)
        tqf = sb("tqf", [PP, SPC * PAD * 4], f32)
        tt = sb("tt", [PP, PPC], f32)
        tgt = sb("tgt", [PP, PPC], f32)
        tfx = sb("tfx", [PP, PPC], f32)
        tfy = sb("tfy", [PP, PPC], f32)
        txc = sb("txc", [PP, PPC], f32)
        tvf = sb("tvf", [PP, PPC], f32)
        ax0 = sb("ax0", [PP, PPC], f32)
        ax1 = sb("ax1", [PP, PPC], f32)
        ay0 = sb("ay0", [PP, PPC], f32)
        ay1 = sb("ay1", [PP, PPC], f32)
        tw = sb("tw", [PP, PPC], f32)
        ts_ = sb("ts", [PP, PPC], f32)
        acc = [sb(f"acc{b}", [PP, PPC], f32) for b in range(2)]
        tvu = [sb(f"tvu{b}", [PP, PPC], u8) for b in range(2)]

        # build geometry: 32 row-blocks x 2 column halves = 64 iterations
        NB = (H - 1 + PP - 1) // PP          # 32 (last block: 127 rows)

        def rows(blk):
            r0 = blk * PP
            return r0, min(PP, (H - 1) - r0)

        # quad table viewed as [image row, 16384 bf16]
        quads_rows = quads_d[:].rearrange("w e -> (w e)") \
                               .rearrange("(y q) -> y q", q=4 * W)
        NG = reps * NCHUNK

        @block.sync
        def _(sync):
            # ---------- phase 1: quad table build ----------
            for s in range(2 * NB):
                blk, h = s // 2, s % 2
                r0, nr = rows(blk)
                b = s % 2
                if s >= 2:
                    sync.wait_ge(p_cp, s - 1)      # ia/ib bufs free
                # overlapped 2049-wide load so the +1-shifted copies stay in
                # tile; the final image column's shifted lanes are never
                # selected (x0 <= 4094), so h==1 skips the boundary column.
                wc = HB + 1 if h == 0 else HB
                sync.dma_start(out=ia[b][:nr, :wc],
                               in_=x_d[r0:r0 + nr, h * HB:h * HB + wc]).then_inc(p_in, 16)
                sync.dma_start(out=ib[b][:nr, :wc],
                               in_=x_d[r0 + 1:r0 + 1 + nr, h * HB:h * HB + wc]).then_inc(p_in, 16)
                sync.wait_ge(p_cp, s + 1)
                sync.dma_start(out=quads_rows[r0:r0 + nr, h * 4 * HB:(h + 1) * 4 * HB],
                               in_=qt[b][:nr, :]).then_inc(p_out, 16)
            sync.wait_ge(p_out, 16 * 2 * NB)

            # ---------- phase 2: main loop ----------
            for g in range(NG):
                k = g % NCHUNK
                if g >= 2:
                    sync.wait_ge(bdone, g - 1)     # in-bufs free
                sync.dma_start(out=twidx[g % 2][:], in_=widx_d[k]).then_inc(cin, 16)
                sync.dma_start(out=txq[g % 2][:], in_=xq_d[k]).then_inc(cin, 16)
                sync.dma_start(out=tyq[g % 2][:], in_=yq_d[k]).then_inc(cin, 16)
                sync.dma_start(out=tm[g % 2][:],
                               in_=m_d[k].rearrange("f p j -> p f j")).then_inc(cin, 16)
                if g >= 2:
                    sync.dma_start(out=val_d[(g - 2) % NCHUNK], in_=acc[g % 2][:]).then_inc(odone, 16)
                    sync.dma_start(out=vld_d[(g - 2) % NCHUNK], in_=tvu[g % 2][:]).then_inc(odone, 16)
            for g in range(max(0, NG - 2), NG):
                sync.wait_ge(bdone, g + 1)
                sync.dma_start(out=val_d[g % NCHUNK], in_=acc[g % 2][:]).then_inc(odone, 16)
                sync.dma_start(out=vld_d[g % NCHUNK], in_=tvu[g % 2][:]).then_inc(odone, 16)
            sync.wait_ge(odone, 32 * NG)

        @block.vector
        def _(vector):
            A = mybir.AluOpType
            # ---------- phase 1: quad interleave ----------
            for s in range(2 * NB):
                blk, h = s // 2, s % 2
                r0, nr = rows(blk)
                b = s % 2
                vector.wait_ge(p_in, 32 * (s + 1))
                if s >= 2:
                    vector.wait_ge(p_out, 16 * (s - 1))   # qt buf free
                vector.tensor_copy(qt[b][:nr, 0:4 * HB:4], ia[b][:nr, 0:HB])
                vector.tensor_copy(qt[b][:nr, 1:4 * HB:4], ia[b][:nr, 1:HB + 1])
                vector.tensor_copy(qt[b][:nr, 2:4 * HB:4], ib[b][:nr, 0:HB])
                vector.tensor_copy(qt[b][:nr, 3:4 * HB:4], ib[b][:nr, 1:HB + 1]) \
                    .then_inc(p_cp, 1)

            # ---------- phase 2 ----------
            def t_view(base, w, off, width):
                return base[:].rearrange("p (s j q) -> p s j q", s=SPC, j=PAD) \
                              [:, :, :, off:off + width]

            def g_view(b, off, width):
                return gt_[b][:].rearrange("p (s q) -> p s q", s=SPC) \
                                [:, :, off:off + width] \
                                .rearrange("p s q -> p s () q") \
                                .to_broadcast([PP, SPC, PAD, width])

            def m_view(b, lvl, width):
                return tm[b][:, lvl * PPC:(lvl + 1) * PPC] \
                    .rearrange("p (s j) -> p s j", s=SPC) \
                    .rearrange("p s j -> p s j ()") \
                    .to_broadcast([PP, SPC, PAD, width])

            for g in range(NG):
                b = g % 2
                vector.wait_ge(cin, 64 * (g + 1))
                if g >= 2:
                    vector.wait_ge(odone, 32 * (g - 1))   # acc/tvu bufs free
                xq, yq = txq[b][:], tyq[b][:]
                # floor(xq) -> tfx ; floor(yq) -> tfy  (baseline magic idiom)
                vector.tensor_scalar_add(tt[:], xq, MAGIC)
                vector.tensor_scalar_sub(tt[:], tt[:], MAGIC)
                vector.tensor_tensor(out=tgt[:], in0=tt[:], in1=xq, op=A.is_gt)
                vector.tensor_tensor(out=tfx[:], in0=tt[:], in1=tgt[:], op=A.subtract)
                vector.tensor_scalar_add(tt[:], yq, MAGIC)
                vector.tensor_scalar_sub(tt[:], tt[:], MAGIC)
                vector.tensor_tensor(out=tgt[:], in0=tt[:], in1=yq, op=A.is_gt)
                vector.tensor_tensor(out=tfy[:], in0=tt[:], in1=tgt[:], op=A.subtract)
                # validity: clamped == unclamped for both axes
                vector.tensor_scalar(txc[:], tfx[:], 0.0, float(W - 2), A.max, A.min)
                vector.tensor_tensor(out=tt[:], in0=txc[:], in1=tfx[:], op=A.is_equal)
                vector.tensor_scalar(txc[:], tfy[:], 0.0, float(H - 2), A.max, A.min)
                vector.tensor_tensor(out=tgt[:], in0=txc[:], in1=tfy[:], op=A.is_equal)
                vector.tensor_tensor(out=tvf[:], in0=tt[:], in1=tgt[:], op=A.mult)
                # weights
                vector.tensor_tensor(out=ax1[:], in0=xq, in1=tfx[:], op=A.subtract)
                vector.tensor_scalar(ax0[:], ax1[:], -1.0, 1.0, A.mult, A.add)
                vector.tensor_tensor(out=ay1[:], in0=yq, in1=tfy[:], op=A.subtract)
                vector.tensor_scalar(ay0[:], ay1[:], -1.0, 1.0, A.mult, A.add)

                # select tree: 5 levels, widths 64,32,16,8,4 (bf16 elems)
                vector.wait_ge(gdone, 16 * SPC * (g + 1))
                vector.tensor_copy(t_view(T1, 64, 0, 64), g_view(b, 0, 64))
                vector.copy_predicated(t_view(T1, 64, 0, 64), m_view(b, 0, 64),
                                       g_view(b, 64, 64)).then_inc(vdone, 1)
                vector.tensor_copy(t_view(T2, 32, 0, 32), t_view(T1, 64, 0, 32))
                vector.copy_predicated(t_view(T2, 32, 0, 32), m_view(b, 1, 32),
                                       t_view(T1, 64, 32, 32))
                vector.tensor_copy(t_view(T3, 16, 0, 16), t_view(T2, 32, 0, 16))
                vector.copy_predicated(t_view(T3, 16, 0, 16), m_view(b, 2, 16),
                                       t_view(T2, 32, 16, 16))
                vector.tensor_copy(t_view(T4, 8, 0, 8), t_view(T3, 16, 0, 8))
                vector.copy_predicated(t_view(T4, 8, 0, 8), m_view(b, 3, 8),
                                       t_view(T3, 16, 8, 8))
                vector.tensor_copy(t_view(T5, 4, 0, 4), t_view(T4, 8, 0, 4))
                vector.copy_predicated(t_view(T5, 4, 0, 4), m_view(b, 4, 4),
                                       t_view(T4, 8, 4, 4))
                vector.tensor_copy(tqf[:], T5[:])      # bf16 -> f32

                # blend (reference accumulation order)
                v00 = tqf[:, 0:4 * PPC:4]
                v10 = tqf[:, 1:4 * PPC:4]
                v01 = tqf[:, 2:4 * PPC:4]
                v11 = tqf[:, 3:4 * PPC:4]
                vector.tensor_tensor(out=tw[:], in0=ax0[:], in1=ay0[:], op=A.mult)
                vector.tensor_tensor(out=acc[b][:], in0=tw[:], in1=v00, op=A.mult)
                vector.tensor_tensor(out=tw[:], in0=ax1[:], in1=ay0[:], op=A.mult)
                vector.tensor_tensor(out=ts_[:], in0=tw[:], in1=v10, op=A.mult)
                vector.tensor_tensor(out=acc[b][:], in0=acc[b][:], in1=ts_[:], op=A.add)
                vector.tensor_tensor(out=tw[:], in0=ax0[:], in1=ay1[:], op=A.mult)
                vector.tensor_tensor(out=ts_[:], in0=tw[:], in1=v01, op=A.mult)
                vector.tensor_tensor(out=acc[b][:], in0=acc[b][:], in1=ts_[:], op=A.add)
                vector.tensor_tensor(out=tw[:], in0=ax1[:], in1=ay1[:], op=A.mult)
                vector.tensor_tensor(out=ts_[:], in0=tw[:], in1=v11, op=A.mult)
                vector.tensor_tensor(out=acc[b][:], in0=acc[b][:], in1=ts_[:], op=A.add)
                vector.tensor_tensor(out=acc[b][:], in0=acc[b][:], in1=tvf[:], op=A.mult)
                vector.tensor_scalar(tvu[b][:], tvf[:], 0.5, None, A.is_ge) \
                    .then_inc(bdone, 1)

        @block.gpsimd
        def _(gpsimd):
            gpsimd.wait_ge(p_out, 16 * 2 * NB)     # quad table complete
            for g in range(NG):
                b = g % 2
                gpsimd.wait_ge(cin, 64 * g + 16)   # widx loaded (first DMA)
                if g >= 2:
                    gpsimd.wait_ge(vdone, g - 1)   # G buf free (tree L1 of g-2)
                for j in range(SPC):
                    gpsimd.indirect_dma_start(
                        out=gt_[b][:, QW * 4 * j:QW * 4 * (j + 1)],
                        out_offset=None,
                        in_=quads_d[:],
                        in_offset=bass.IndirectOffsetOnAxis(
                            ap=twidx[b][:, j:j + 1], axis=0),
                    ).then_inc(gdone, 16)

    return nc


def prepare(x, coords):
    """Host-side packing. Returns (in_maps, unsort) where unsort holds the
    per-core (order, devpos) needed to reassemble full outputs."""
    x = np.ascontiguousarray(np.asarray(x), dtype=np.float32)
    coords = np.asarray(coords, dtype=np.float32)
    in_maps, unsort = [], []
    for cidx in range(NCORES):
        sl = slice(cidx * NPC, (cidx + 1) * NPC)
        xq = np.ascontiguousarray(coords[0, sl])
        yq = np.ascontiguousarray(coords[1, sl])
        fx = np.floor(xq)
        fy = np.floor(yq)
        xc = np.clip(fx, 0, W - 2).astype(np.int64)
        yc = np.clip(fy, 0, H - 2).astype(np.int64)
        f = yc * W + xc
        w = (f >> 5).astype(np.int32)
        c = (f & 31).astype(np.uint8)

        order = np.argsort(w, kind="stable")
        ws = w[order]
        bounds = np.flatnonzero(ws[1:] != ws[:-1]) + 1
        gs = np.concatenate(([0], bounds))                  # group starts
        gn = np.diff(np.concatenate((gs, [NPC])))           # group counts
        spw = -(-gn // PAD)                                 # slots per window
        sbase = np.concatenate(([0], np.cumsum(spw)))
        total_slots = int(sbase[-1])
        spill = None
        if total_slots > NSLOT:
            raise RuntimeError(
                f"slot overflow: {total_slots} > {NSLOT}; increase NCHUNK")
        rank = np.arange(NPC, dtype=np.int64) - np.repeat(gs, gn)
        slot = np.repeat(sbase[:-1], gn) + (rank >> 3)
        pos = rank & 7
        kk = slot >> 12
        rr = slot & 4095
        part = rr & 127
        j = rr >> 7
        devpos = (kk * PP + part) * PPC + j * PAD + pos     # into [NCHUNK,PP,PPC]

        xq_d = np.full(NCHUNK * PP * PPC, -100.0, np.float32)
        yq_d = np.full(NCHUNK * PP * PPC, -100.0, np.float32)
        xq_d[devpos] = xq[order]
        yq_d[devpos] = yq[order]

        cs = c[order]
        m_d = np.zeros((NCHUNK, 5, PP * PPC), np.uint8)
        within = (kk * PP + part) * PPC + j * PAD + pos - kk * (PP * PPC)
        for lvl, bit in enumerate((4, 3, 2, 1, 0)):
            plane = ((cs >> bit) & 1).astype(np.uint8)
            m_d[kk, lvl, within] = plane

        wslot = np.zeros(NSLOT, np.int32)
        wslot[slot] = ws
        sidx = np.arange(NSLOT, dtype=np.int64)
        skk = sidx >> 12
        srr = sidx & 4095
        spp = srr & 127
        sjj = srr >> 7
        widx_d = np.zeros(NCHUNK * PP * SPC, np.int32)
        widx_d[(skk * PP + spp) * SPC + sjj] = wslot

        in_maps.append({
            "x": x,
            "widx": widx_d.reshape(NCHUNK, PP, SPC),
            "xq": xq_d.reshape(NCHUNK, PP, PPC),
            "yq": yq_d.reshape(NCHUNK, PP, PPC),
            "m": m_d.reshape(NCHUNK, 5, PP, PPC),
        })
        unsort.append((order, devpos))
    return in_maps, unsort


def postprocess(results, unsort):
    values = np.empty(N, np.float32)
    valid = np.empty(N, bool)
    for cidx in range(NCORES):
        order, devpos = unsort[cidx]
        vdev = results[cidx]["values"].reshape(-1)
        udev = results[cidx]["valid"].reshape(-1)
        sl = slice(cidx * NPC, (cidx + 1) * NPC)
        vals = np.empty(NPC, np.float32)
        vals[order] = vdev[devpos]
        vld = np.empty(NPC, np.uint8)
        vld[order] = udev[devpos]
        values[sl] = vals
        valid[sl] = vld.astype(bool)
    return values, valid


_nc_cache = None


def kernel(x: np.ndarray, coords: np.ndarray):
    global _nc_cache
    if _nc_cache is None:
        _nc_cache = build_nc()
    in_maps, unsort = prepare(x, coords)
    res = run_bass_kernel_spmd(_nc_cache, in_maps, list(range(NCORES)))
    return postprocess(res.results, unsort)


# revision 11
# speedup vs baseline: 1.9167x; 1.9167x over previous
"""Bilinear interpolation kernel for Trainium2 (8 NeuronCores, SPMD).

Strategy — "window-shared fat gathers":
  The per-point random gather is the bottleneck: SWDGE indirect DMA costs
  ~1-1.5us PER INSTRUCTION (128 descriptors max, one per partition), so the
  old one-16B-descriptor-per-point scheme is pinned at N/128 instructions.
  Instead:

  * Device builds a bf16 QUAD table: quad[f] = the 4 bilinear corners of
    flat position f = y*4096+x, 8 bytes each.  256B "windows" of the table
    cover 32 consecutive quads.
  * Host sorts each core's points by window id and packs up to PAD=8 points
    per occupied window into one "slot".  One indirect-DMA descriptor
    fetches one 256B window per partition -> 128 windows (up to 1024
    points) per instruction: ~4.1k gather instructions/core instead of 16k.
  * Each point selects its quad (position f mod 32 inside the window) on
    the Vector engine with a 5-level binary select tree (tensor_copy +
    copy_predicated with host-provided u8 mask planes), then the usual
    fp32 weight/blend/mask math.
  * Host un-permutes the outputs.

  Weights, validity and the blend are computed on-device in fp32 exactly as
  the reference; only the 4 corner values pass through bf16 (rel err ~4e-3,
  well inside the 2e-2 gate).
"""
import sys

sys.path.insert(0, "/opt/trn_rl_repo")

from contextlib import ExitStack

import numpy as np

import concourse.bass as bass
import concourse.mybir as mybir
from concourse.bass_utils import run_bass_kernel_spmd

H = W = 4096
N = 16777216
NCORES = 8
NPC = N // NCORES            # points per core (2_097_152)
PP = 128                     # partitions

QW = 32                      # quads per 256B window (bf16 quad = 8B)
PAD = 8                      # point positions per window slot
SPC = 32                     # slots per partition per chunk
PPC = SPC * PAD              # point columns per partition per chunk (256)
SLOTC = PP * SPC             # slots per chunk (4096)
NCHUNK = 129
NSLOT = NCHUNK * SLOTC       # 528_384 slots (mean occupancy ~525.8k)
NFLAT = (H - 1) * W          # quad table entries (16_773_120)
NWIN = NFLAT // QW           # windows (524_160)
MAGIC = 8388608.0            # 2^23 fp32 round-to-int magic

f32 = mybir.dt.float32
bf16 = mybir.dt.bfloat16
i32 = mybir.dt.int32
u8 = mybir.dt.uint8


def build_nc(reps=1):
    nc = bass.Bass()
    x_d = nc.declare_dram_parameter("x", [H, W], f32, isOutput=False)
    widx_d = nc.declare_dram_parameter("widx", [NCHUNK, PP, SPC], i32, isOutput=False)
    xq_d = nc.declare_dram_parameter("xq", [NCHUNK, PP, PPC], f32, isOutput=False)
    yq_d = nc.declare_dram_parameter("yq", [NCHUNK, PP, PPC], f32, isOutput=False)
    m_d = nc.declare_dram_parameter("m", [NCHUNK, 5, PP, PPC], u8, isOutput=False)
    val_d = nc.declare_dram_parameter("values", [NCHUNK, PP, PPC], f32, isOutput=True)
    vld_d = nc.declare_dram_parameter("valid", [NCHUNK, PP, PPC], u8, isOutput=True)
    quads_d = nc.dram_tensor("quads", [NWIN, QW * 4], bf16)

    es = ExitStack()
    with (
        nc.semaphore("p_in") as p_in,      # build: img tiles loaded
        nc.semaphore("p_cp") as p_cp,      # build: interleave done (1/iter)
        nc.semaphore("p_out") as p_out,    # build: quad tile stored
        nc.semaphore("cin") as cin,        # chunk inputs loaded (64/chunk)
        nc.semaphore("gdone") as gdone,    # gathers done (16/chunk)
        nc.semaphore("vdone") as vdone,    # tree L1 done -> G buf free (1/chunk)
        nc.semaphore("bdone") as bdone,    # blend done (1/chunk)
        nc.semaphore("odone") as odone,    # outputs stored (32/chunk)
        nc.Block() as block,
        es,
    ):
        def sb(name, shape, dt):
            return es.enter_context(nc.sbuf_tensor(name, shape, dt))

        HB = W // 2                     # build column half (2048)
        # build tiles
        ia = [sb(f"ia{b}", [PP, HB + 1], f32) for b in range(2)]
        ib = [sb(f"ib{b}", [PP, HB + 1], f32) for b in range(2)]
        qt = [sb(f"qt{b}", [PP, 4 * HB], bf16) for b in range(2)]
        # main loop tiles
        twidx = [sb(f"twidx{b}", [PP, SPC], i32) for b in range(2)]
        txq = [sb(f"txq{b}", [PP, PPC], f32) for b in range(2)]
        tyq = [sb(f"tyq{b}", [PP, PPC], f32) for b in range(2)]
        tm = [sb(f"tm{b}", [PP, 5 * PPC], u8) for b in range(2)]
        gt_ = [sb(f"g{b}", [PP, SPC * QW * 4], bf16) for b in range(2)]
        T1 = sb("T1", [PP, SPC * PAD * QW * 2], bf16)
        T2 = sb("T2", [PP, SPC * PAD * QW], bf16)
        T3 = sb("T3", [PP, SPC * PAD * QW // 2], bf16)
        T4 = sb("T4", [PP, SPC * PAD * QW // 4], bf16)
        T5 = sb("T5", [PP, SPC * PAD * 4], bf16)
        tqf = sb("tqf", [PP, SPC * PAD * 4], f32)
        tt = sb("tt", [PP, PPC], f32)
        tgt = sb("tgt", [PP, PPC], f32)
        tfx = sb("tfx", [PP, PPC], f32)
        tfy = sb("tfy", [PP, PPC], f32)
        txc = sb("txc", [PP, PPC], f32)
        tvf = sb("tvf", [PP, PPC], f32)
        ax0 = sb("ax0", [PP, PPC], f32)
        ax1 = sb("ax1", [PP, PPC], f32)
        ay0 = sb("ay0", [PP, PPC], f32)
        ay1 = sb("ay1", [PP, PPC], f32)
        tw = sb("tw", [PP, PPC], f32)
        ts_ = sb("ts", [PP, PPC], f32)
        acc = [sb(f"acc{b}", [PP, PPC], f32) for b in range(2)]
        tvu = [sb(f"tvu{b}", [PP, PPC], u8) for b in range(2)]

        # build geometry: 32 row-blocks x 2 column halves = 64 iterations
        NB = (H - 1 + PP - 1) // PP          # 32 (last block: 127 rows)

        def rows(blk):
            r0 = blk * PP
            return r0, min(PP, (H - 1) - r0)

        # quad table viewed as [image row, 16384 bf16]
        quads_rows = quads_d[:].rearrange("w e -> (w e)") \
                               .rearrange("(y q) -> y q", q=4 * W)
        NG = reps * NCHUNK

        @block.sync
        def _(sync):
            # ---------- phase 1: quad table build ----------
            for s in range(2 * NB):
                blk, h = s // 2, s % 2
                r0, nr = rows(blk)
                b = s % 2
                if s >= 2:
                    sync.wait_ge(p_cp, s - 1)      # ia/ib bufs free
                # overlapped 2049-wide load so the +1-shifted copies stay in
                # tile; the final image column's shifted lanes are never
                # selected (x0 <= 4094), so h==1 skips the boundary column.
                wc = HB + 1 if h == 0 else HB
                sync.dma_start(out=ia[b][:nr, :wc],
                               in_=x_d[r0:r0 + nr, h * HB:h * HB + wc]).then_inc(p_in, 16)
                sync.dma_start(out=ib[b][:nr, :wc],
                               in_=x_d[r0 + 1:r0 + 1 + nr, h * HB:h * HB + wc]).then_inc(p_in, 16)
                sync.wait_ge(p_cp, s + 1)
                sync.dma_start(out=quads_rows[r0:r0 + nr, h * 4 * HB:(h + 1) * 4 * HB],
                               in_=qt[b][:nr, :]).then_inc(p_out, 16)
            sync.wait_ge(p_out, 16 * 2 * NB)

            # ---------- phase 2: main loop ----------
            for g in range(NG):
                k = g % NCHUNK
                if g >= 2:
                    sync.wait_ge(bdone, g - 1)     # in-bufs free
                sync.dma_start(out=twidx[g % 2][:], in_=widx_d[k]).then_inc(cin, 16)
                sync.dma_start(out=txq[g % 2][:], in_=xq_d[k]).then_inc(cin, 16)
                sync.dma_start(out=tyq[g % 2][:], in_=yq_d[k]).then_inc(cin, 16)
                sync.dma_start(out=tm[g % 2][:],
                               in_=m_d[k].rearrange("f p j -> p f j")).then_inc(cin, 16)
                if g >= 2:
                    sync.dma_start(out=val_d[(g - 2) % NCHUNK], in_=acc[g % 2][:]).then_inc(odone, 16)
                    sync.dma_start(out=vld_d[(g - 2) % NCHUNK], in_=tvu[g % 2][:]).then_inc(odone, 16)
            for g in range(max(0, NG - 2), NG):
                sync.wait_ge(bdone, g + 1)
                sync.dma_start(out=val_d[g % NCHUNK], in_=acc[g % 2][:]).then_inc(odone, 16)
                sync.dma_start(out=vld_d[g % NCHUNK], in_=tvu[g % 2][:]).then_inc(odone, 16)
            sync.wait_ge(odone, 32 * NG)

        @block.vector
        def _(vector):
            A = mybir.AluOpType
            # ---------- phase 1: quad interleave ----------
            for s in range(2 * NB):
                blk, h = s // 2, s % 2
                r0, nr = rows(blk)
                b = s % 2
                vector.wait_ge(p_in, 32 * (s + 1))
                if s >= 2:
                    vector.wait_ge(p_out, 16 * (s - 1))   # qt buf free
                vector.tensor_copy(qt[b][:nr, 0:4 * HB:4], ia[b][:nr, 0:HB])
                vector.tensor_copy(qt[b][:nr, 1:4 * HB:4], ia[b][:nr, 1:HB + 1])
                vector.tensor_copy(qt[b][:nr, 2:4 * HB:4], ib[b][:nr, 0:HB])
                vector.tensor_copy(qt[b][:nr, 3:4 * HB:4], ib[b][:nr, 1:HB + 1]) \
                    .then_inc(p_cp, 1)

            # ---------- phase 2 ----------
            def t_view(base, w, off, width):
                return base[:].rearrange("p (s j q) -> p s j q", s=SPC, j=PAD) \
                              [:, :, :, off:off + width]

            def g_view(b, off, width):
                return gt_[b][:].rearrange("p (s q) -> p s q", s=SPC) \
                                [:, :, off:off + width] \
                                .rearrange("p s q -> p s () q") \
                                .to_broadcast([PP, SPC, PAD, width])

            def m_view(b, lvl, width):
                return tm[b][:, lvl * PPC:(lvl + 1) * PPC] \
                    .rearrange("p (s j) -> p s j", s=SPC) \
                    .rearrange("p s j -> p s j ()") \
                    .to_broadcast([PP, SPC, PAD, width])

            for g in range(NG):
                b = g % 2
                vector.wait_ge(cin, 64 * (g + 1))
                if g >= 2:
                    vector.wait_ge(odone, 32 * (g - 1))   # acc/tvu bufs free
                xq, yq = txq[b][:], tyq[b][:]
                # floor(xq) -> tfx ; floor(yq) -> tfy  (baseline magic idiom)
                vector.tensor_scalar_add(tt[:], xq, MAGIC)
                vector.tensor_scalar_sub(tt[:], tt[:], MAGIC)
                vector.tensor_tensor(out=tgt[:], in0=tt[:], in1=xq, op=A.is_gt)
                vector.tensor_tensor(out=tfx[:], in0=tt[:], in1=tgt[:], op=A.subtract)
                vector.tensor_scalar_add(tt[:], yq, MAGIC)
                vector.tensor_scalar_sub(tt[:], tt[:], MAGIC)
                vector.tensor_tensor(out=tgt[:], in0=tt[:], in1=yq, op=A.is_gt)
                vector.tensor_tensor(out=tfy[:], in0=tt[:], in1=tgt[:], op=A.subtract)
                # validity: clamped == unclamped for both axes
                vector.tensor_scalar(txc[:], tfx[:], 0.0, float(W - 2), A.max, A.min)
                vector.tensor_tensor(out=tt[:], in0=txc[:], in1=tfx[:], op=A.is_equal)
                vector.tensor_scalar(txc[:], tfy[:], 0.0, float(H - 2), A.max, A.min)
                vector.tensor_tensor(out=tgt[:], in0=txc[:], in1=tfy[:], op=A.is_equal)
                vector.tensor_tensor(out=tvf[:], in0=tt[:], in1=tgt[:], op=A.mult)
                # weights
                vector.tensor_tensor(out=ax1[:], in0=xq, in1=tfx[:], op=A.subtract)
                vector.tensor_scalar(ax0[:], ax1[:], -1.0, 1.0, A.mult, A.add)
                vector.tensor_tensor(out=ay1[:], in0=yq, in1=tfy[:], op=A.subtract)
                vector.tensor_scalar(ay0[:], ay1[:], -1.0, 1.0, A.mult, A.add)

                # select tree: 5 levels, widths 64,32,16,8,4 (bf16 elems)
                vector.wait_ge(gdone, 16 * SPC * (g + 1))
                vector.tensor_copy(t_view(T1, 64, 0, 64), g_view(b, 0, 64))
                vector.copy_predicated(t_view(T1, 64, 0, 64), m_view(b, 0, 64),
                                       g_view(b, 64, 64)).then_inc(vdone, 1)
                vector.tensor_copy(t_view(T2, 32, 0, 32), t_view(T1, 64, 0, 32))
                vector.copy_predicated(t_view(T2, 32, 0, 32), m_view(b, 1, 32),
                                       t_view(T1, 64, 32, 32))
                vector.tensor_copy(t_view(T3, 16, 0, 16), t_view(T2, 32, 0, 16))
                vector.copy_predicated(t_view(T3, 16, 0, 16), m_view(b, 2, 16),
                                       t_view(T2, 32, 16, 16))
                vector.tensor_copy(t_view(T4, 8, 0, 8), t_view(T3, 16, 0, 8))
                vector.copy_predicated(t_view(T4, 8, 0, 8), m_view(b, 3, 8),
                                       t_view(T3, 16, 8, 8))
                vector.tensor_copy(t_view(T5, 4, 0, 4), t_view(T4, 8, 0, 4))
                vector.copy_predicated(t_view(T5, 4, 0, 4), m_view(b, 4, 4),
                                       t_view(T4, 8, 4, 4))
                vector.tensor_copy(tqf[:], T5[:])      # bf16 -> f32

                # blend (reference accumulation order)
                v00 = tqf[:, 0:4 * PPC:4]
                v10 = tqf[:, 1:4 * PPC:4]
                v01 = tqf[:, 2:4 * PPC:4]
                v11 = tqf[:, 3:4 * PPC:4]
                vector.tensor_tensor(out=tw[:], in0=ax0[:], in1=ay0[:], op=A.mult)
                vector.tensor_tensor(out=acc[b][:], in0=tw[:], in1=v00, op=A.mult)
                vector.tensor_tensor(out=tw[:], in0=ax1[:], in1=ay0[:], op=A.mult)
                vector.tensor_tensor(out=ts_[:], in0=tw[:], in1=v10, op=A.mult)
                vector.tensor_tensor(out=acc[b][:], in0=acc[b][:], in1=ts_[:], op=A.add)
                vector.tensor_tensor(out=tw[:], in0=ax0[:], in1=ay1[:], op=A.mult)
                vector.tensor_tensor(out=ts_[:], in0=tw[:], in1=v01, op=A.mult)
                vector.tensor_tensor(out=acc[b][:], in0=acc[b][:], in1=ts_[:], op=A.add)
                vector.tensor_tensor(out=tw[:], in0=ax1[:], in1=ay1[:], op=A.mult)
                vector.tensor_tensor(out=ts_[:], in0=tw[:], in1=v11, op=A.mult)
                vector.tensor_tensor(out=acc[b][:], in0=acc[b][:], in1=ts_[:], op=A.add)
                vector.tensor_tensor(out=acc[b][:], in0=acc[b][:], in1=tvf[:], op=A.mult)
                vector.tensor_scalar(tvu[b][:], tvf[:], 0.5, None, A.is_ge) \
                    .then_inc(bdone, 1)

        @block.gpsimd
        def _(gpsimd):
            gpsimd.wait_ge(p_out, 16 * 2 * NB)     # quad table complete
            for g in range(NG):
                b = g % 2
                gpsimd.wait_ge(cin, 64 * g + 16)   # widx loaded (first DMA)
                if g >= 2:
                    gpsimd.wait_ge(vdone, g - 1)   # G buf free (tree L1 of g-2)
                for j in range(SPC):
                    gpsimd.indirect_dma_start(
                        out=gt_[b][:, QW * 4 * j:QW * 4 * (j + 1)],
                        out_offset=None,
                        in_=quads_d[:],
                        in_offset=bass.IndirectOffsetOnAxis(
                            ap=twidx[b][:, j:j + 1], axis=0),
                    ).then_inc(gdone, 16)

    return nc


def prepare(x, coords):
    """Host-side packing. Returns (in_maps, unsort) where unsort holds the
    per-core (order, devpos) needed to reassemble full outputs."""
    x = np.ascontiguousarray(np.asarray(x), dtype=np.float32)
    coords = np.asarray(coords, dtype=np.float32)
    in_maps, unsort = [], []
    for cidx in range(NCORES):
        sl = slice(cidx * NPC, (cidx + 1) * NPC)
        xq = np.ascontiguousarray(coords[0, sl])
        yq = np.ascontiguousarray(coords[1, sl])
        fx = np.floor(xq)
        fy = np.floor(yq)
        xc = np.clip(fx, 0, W - 2).astype(np.int64)
        yc = np.clip(fy, 0, H - 2).astype(np.int64)
        f = yc * W + xc
        w = (f >> 5).astype(np.int32)
        c = (f & 31).astype(np.uint8)

        order = np.argsort(w, kind="stable")
        ws = w[order]
        bounds = np.flatnonzero(ws[1:] != ws[:-1]) + 1
        gs = np.concatenate(([0], bounds))                  # group starts
        gn = np.diff(np.concatenate((gs, [NPC])))           # group counts
        spw = -(-gn // PAD)                                 # slots per window
        sbase = np.concatenate(([0], np.cumsum(spw)))
        total_slots = int(sbase[-1])
        rank = np.arange(NPC, dtype=np.int64) - np.repeat(gs, gn)
        slot = np.repeat(sbase[:-1], gn) + (rank >> 3)
        pos = rank & 7
        ws_pts = ws
        # overflow safety net (never hit for the reference inputs): points
        # whose slot spills past NSLOT are computed on the host instead.
        spill = None
        if total_slots > NSLOT:
            sm = slot >= NSLOT
            sidx_orig = order[sm]
            sxq, syq = xq[sidx_orig], yq[sidx_orig]
            sfx, sfy = np.floor(sxq), np.floor(syq)
            sxc = np.clip(sfx, 0, W - 2).astype(np.int64)
            syc = np.clip(sfy, 0, H - 2).astype(np.int64)
            sval = ((sfx >= 0) & (sfx <= W - 2) & (sfy >= 0) & (sfy <= H - 2))
            flat = x.reshape(-1)
            v00 = flat[syc * W + sxc]
            v10 = flat[syc * W + sxc + 1]
            v01 = flat[(syc + 1) * W + sxc]
            v11 = flat[(syc + 1) * W + sxc + 1]
            a1, b1 = sxq - sfx, syq - sfy
            a0, b0 = 1.0 - a1, 1.0 - b1
            svals = (a0 * b0 * v00 + a1 * b0 * v10 + a0 * b1 * v01
                     + a1 * b1 * v11) * sval
            spill = (sidx_orig, svals.astype(np.float32), sval)
            keep = ~sm
            order, slot, pos, ws_pts = order[keep], slot[keep], pos[keep], ws[keep]
        kk = slot >> 12
        rr = slot & 4095
        part = rr & 127
        j = rr >> 7
        devpos = (kk * PP + part) * PPC + j * PAD + pos     # into [NCHUNK,PP,PPC]

        xq_d = np.full(NCHUNK * PP * PPC, -100.0, np.float32)
        yq_d = np.full(NCHUNK * PP * PPC, -100.0, np.float32)
        xq_d[devpos] = xq[order]
        yq_d[devpos] = yq[order]

        cs = c[order]
        m_d = np.zeros((NCHUNK, 5, PP * PPC), np.uint8)
        within = (kk * PP + part) * PPC + j * PAD + pos - kk * (PP * PPC)
        for lvl, bit in enumerate((4, 3, 2, 1, 0)):
            plane = ((cs >> bit) & 1).astype(np.uint8)
            m_d[kk, lvl, within] = plane

        wslot = np.zeros(NSLOT, np.int32)
        wslot[slot] = ws_pts
        sidx = np.arange(NSLOT, dtype=np.int64)
        skk = sidx >> 12
        srr = sidx & 4095
        spp = srr & 127
        sjj = srr >> 7
        widx_d = np.zeros(NCHUNK * PP * SPC, np.int32)
        widx_d[(skk * PP + spp) * SPC + sjj] = wslot

        in_maps.append({
            "x": x,
            "widx": widx_d.reshape(NCHUNK, PP, SPC),
            "xq": xq_d.reshape(NCHUNK, PP, PPC),
            "yq": yq_d.reshape(NCHUNK, PP, PPC),
            "m": m_d.reshape(NCHUNK, 5, PP, PPC),
        })
        unsort.append((order, devpos, spill))
    return in_maps, unsort


def postprocess(results, unsort):
    values = np.empty(N, np.float32)
    valid = np.empty(N, bool)
    for cidx in range(NCORES):
        order, devpos, spill = unsort[cidx]
        vdev = results[cidx]["values"].reshape(-1)
        udev = results[cidx]["valid"].reshape(-1)
        sl = slice(cidx * NPC, (cidx + 1) * NPC)
        vals = np.empty(NPC, np.float32)
        vals[order] = vdev[devpos]
        vld = np.empty(NPC, np.uint8)
        vld[order] = udev[devpos]
        if spill is not None:
            sidx_orig, svals, sval = spill
            vals[sidx_orig] = svals
            vld[sidx_orig] = sval.astype(np.uint8)
        values[sl] = vals
        valid[sl] = vld.astype(bool)
    return values, valid


_nc_cache = None


def kernel(x: np.ndarray, coords: np.ndarray):
    global _nc_cache
    if _nc_cache is None:
        _nc_cache = build_nc()
    in_maps, unsort = prepare(x, coords)
    res = run_bass_kernel_spmd(_nc_cache, in_maps, list(range(NCORES)))
    return postprocess(res.results, unsort)
